# revision 12
# baseline (speedup 1.0000x reference)
"""Trainium2 Bass kernel for nn_AnisotropyFrame (GNN anisotropy message passing).

Strategy (8 NeuronCores, no collectives):
  - Shard edges by ROW-RANGE: core k owns nodes [k*6250, (k+1)*6250); every
    scatter target is core-local.
  - Phase 0 (device): g = h @ W1[:128] + curv @ W1[128:132] + b1 per local
    node, packed (bf16) with the node's xyz into a 256B-stride "gx" HBM table.
  - Phase 1 (device), edges sorted by row and grouped into 128-node windows,
    padded to a uniform G groups of 128 edges per window (SPMD-uniform):
      dma_gather gx[row]        (136B bf16 rows: g | x_row)
      dma_gather xpair[col>>1]  (16B bf16: two nodes' xyz; parity-select)
      edge-major DVE/ACT pipeline: dist, silu MLP, alpha, uv=alpha*diff/dist
      scatter via PE: one-hot M (built by DVE iota-compare) matmuls
      accumulate uv into PSUM [128-node-window, 4], flushed to an
      SBUF-resident accumulator D.
  - Phase 2 (device): normalize D rows -> out.

dma_scatter_add is NOT used (CCE-add collisions lose updates on this
runtime); dma_gather requires single_packet=False for >1024 indices.

kernel(**inputs) takes FULL unsharded inputs, returns FULL (50000,3) float32.
"""

import os
import sys
from contextlib import ExitStack
from dataclasses import dataclass

import numpy as np
import ml_dtypes

sys.path.insert(0, "/opt/trn_rl_repo")

import concourse.bass as bass
import concourse.mybir as mybir
import concourse.tile as tile
from concourse import bacc
from concourse import ap_utils
from concourse.bass_utils import run_bass_kernel_spmd

F32 = mybir.dt.float32
BF16 = mybir.dt.bfloat16
I16 = mybir.dt.int16
AF = mybir.ActivationFunctionType
ALU = mybir.AluOpType
NPBF = ml_dtypes.bfloat16

NCORES = 8
HID = 128
CURV = 4
MID = 64
EPS2 = 1e-12  # EPS**2


@dataclass(frozen=True)
class Cfg:
    V: int          # total nodes
    VPC: int        # nodes per core
    VPAD: int       # VPC padded to 128 multiple
    RTP: int        # xpair table rows (>= ceil(V/2))
    G: int          # uniform groups (of 128 edges) per 64-node window

    @property
    def nwin(self):
        return self.VPAD // 64

    @property
    def nchunk(self):
        return self.VPAD // 128


# ------------------------------------------------------- raw gather builder

def dma_gather_raw(gp, out_ap, in_ap, idxs_ap, num_idxs, num_idxs_reg,
                   elem_size, elem_step=None, single_packet=True,
                   queue_num=0):
    """bass.dma_gather minus the elem_size%256B assert (the ISA only requires
    the table STRIDE to be a 256B multiple; elem_size is free)."""
    from concourse.bass import MemorySpace
    from concourse._compat import exact_div

    assert idxs_ap.dtype == mybir.dt.int16
    assert in_ap.dtype == out_ap.dtype
    assert in_ap.space == MemorySpace.DRAM
    assert idxs_ap.space == MemorySpace.SBUF
    assert out_ap.space == MemorySpace.SBUF
    if elem_step is None:
        assert ap_utils.ap_is_contiguous(in_ap.ap[1:])
        elem_step = elem_size
    assert ap_utils.ap_is_contiguous(out_ap.ap[1:])
    assert ap_utils.ap_is_contiguous(idxs_ap.ap[1:])
    assert in_ap.ap[-1][1] == out_ap.ap[-1][1] == elem_size
    assert out_ap.ap[0][1] * out_ap.ap[1][1] == ((num_idxs + 127) // 128) * 128
    assert in_ap.ap[0][0] == elem_step
    stride_bytes = elem_step * mybir.dt.size(in_ap.dtype)
    stride_bytes_256 = exact_div(stride_bytes, 256)
    assert stride_bytes_256 < 256
    _in_ap = gp.lower_ap_dma(in_ap, for_custom_bir_dma=True)
    _idxs_ap = gp.lower_ap(idxs_ap)
    _out_ap = gp.lower_ap(out_ap)
    return gp.add_instruction(
        mybir.InstDMAGatherAnt(
            name=gp.bass.get_next_instruction_name(),
            ins=[*_in_ap, _idxs_ap,
                 gp.lower_val_access(gp.to_reg(num_idxs_reg))],
            outs=[_out_ap],
            transpose=False,
            num_idxs=num_idxs,
            elem_size=elem_size,
            stride_bytes_256=stride_bytes_256,
            gen_mode=0,
            single_packet=single_packet,
            queue_num=queue_num,
            sbuf_tokens_per_rank=0,
            sbuf_free_dim_per_rank=0,
            sbuf_free_dim_pad_per_rank=0,
            sbuf_byte_offset=0,
        ))


# ---------------------------------------------------------------- host prep

def _wrap_idx(flat):
    """Slot-ordered int16 [N] -> wrapped [128, N/16] (idx i at (i%16, i//16),
    replicated across the 8 Q7 cores)."""
    w = flat.reshape(-1, 16).T  # [16, N/16]
    return np.ascontiguousarray(np.tile(w, (8, 1)))


def prep_inputs(h, x, curvature, edge_index, W1, b1, W2, b2, cfg):
    V, VPC, VPAD = cfg.V, cfg.VPC, cfg.VPAD
    row = np.asarray(edge_index[0], np.int64)
    col = np.asarray(edge_index[1], np.int64)
    h = np.asarray(h, np.float32)
    x = np.asarray(x, np.float32)
    curvature = np.asarray(curvature, np.float32)
    W1 = np.asarray(W1, np.float32)
    b1 = np.asarray(b1, np.float32)
    W2 = np.asarray(W2, np.float32)
    b2val = float(np.asarray(b2, np.float32).reshape(-1)[0])
    nwin = cfg.nwin

    x4 = np.zeros((V + 2, 4), np.float32)
    x4[:V, :3] = x

    # per-core edge partition, sorted by local row
    cores = []
    maxfill = 0
    for k in range(NCORES):
        base = k * VPC
        m = (row >= base) & (row < base + VPC)
        rl = (row[m] - base).astype(np.int32)
        cg = col[m].astype(np.int32)
        o = np.argsort(rl, kind="stable")
        rl, cg = rl[o], cg[o]
        w = rl >> 6
        fills = np.bincount(w, minlength=nwin)
        maxfill = max(maxfill, int(fills.max()))
        cores.append((base, rl, cg, fills))
    G = max(1, -(-maxfill // 128))
    cfg2 = Cfg(cfg.V, cfg.VPC, cfg.VPAD, cfg.RTP, G)
    BLKE = G * 128

    # shared tensors
    W1a = np.ascontiguousarray(W1[0:HID])                       # [128, 64]
    W1b = np.concatenate([W1[HID:HID + CURV], b1[None, :]], 0)  # [5, 64]
    w1dt = np.tile(W1[HID + CURV][None, :], (128, 1)).astype(np.float32)
    W2t = np.tile(W2[:, 0][None, :], (128, 1)).astype(np.float32)
    iotaF = np.tile(np.arange(64, dtype=np.float32)[None, :],
                    (128, 1)).astype(NPBF)
    xpair = np.zeros((cfg.RTP, 64), np.float32)
    npair = (V + 1) // 2
    xpair[:npair, 0:8] = x4[:2 * npair].reshape(npair, 8)

    in_maps = []
    for k in range(NCORES):
        base, rl, cg, fills = cores[k]
        rli = np.empty((nwin, BLKE), np.int16)
        cpi = np.empty((nwin, BLKE), np.int16)
        par = np.empty((nwin, BLKE), np.float32)
        rrel = np.empty((nwin, BLKE), NPBF)
        pos = 0
        for w in range(nwin):
            n = int(fills[w])
            rl_w = rl[pos:pos + n]
            cg_w = cg[pos:pos + n]
            pos += n
            dummy_col = base + 64 * w
            rr = np.full(BLKE, 64 * w, np.int32)
            cc = np.full(BLKE, dummy_col, np.int32)
            rr[:n] = rl_w
            cc[:n] = cg_w
            rli[w] = rr.astype(np.int16)
            cpi[w] = (cc >> 1).astype(np.int16)
            par[w] = (cc & 1).astype(np.float32)
            rrel[w] = (rr - 64 * w).astype(NPBF)
        rli_w = np.stack([_wrap_idx(rli[w]) for w in range(nwin)])
        cpi_w = np.stack([_wrap_idx(cpi[w]) for w in range(nwin)])
        # compute arrays in edge-major [128, G] layout (slot i = p + 128*g)
        par_pg = np.ascontiguousarray(
            par.reshape(nwin, G, 128).transpose(0, 2, 1))
        rrel_pg = np.ascontiguousarray(
            rrel.reshape(nwin, G, 128).transpose(0, 2, 1))

        hT = np.zeros((HID, VPAD), np.float32)
        hT[:, :VPC] = h[base:base + VPC].T
        cT1 = np.zeros((5, VPAD), np.float32)
        cT1[:CURV, :VPC] = curvature[base:base + VPC].T
        cT1[CURV, :] = 1.0
        x4r = np.zeros((VPAD, 4), np.float32)
        x4r[:VPC] = x4[base:base + VPC]

        in_maps.append({
            "hT": hT, "cT1": cT1, "W1a": W1a, "W1b": W1b,
            "w1dt": w1dt, "W2t": W2t, "iotaF": iotaF,
            "x4r": x4r, "xpair": xpair,
            "rli": rli_w, "cpi": cpi_w, "par": par_pg, "rrel": rrel_pg,
        })
    return in_maps, cfg2, b2val


# ---------------------------------------------------------------- graph build

def build_graph(cfg, b2val, num_devices=NCORES):
    VPAD, RTP, G = cfg.VPAD, cfg.RTP, cfg.G
    nwin = cfg.nwin          # 64-node windows
    nch = cfg.nchunk         # 128-node phase-0 chunks
    BLKE = G * 128           # edge slots per window
    SBW = 4                  # windows per super-block (gather granularity)
    while nwin % SBW != 0:
        SBW -= 1
    NSB = nwin // SBW
    SBE = SBW * BLKE         # edge slots per super-block
    CS = 8                   # MLP sub-chunk columns

    nc = bacc.Bacc("TRN2", target_bir_lowering=False, debug=False,
                   enable_asserts=False, num_devices=num_devices)

    hT = nc.dram_tensor("hT", [HID, VPAD], F32, kind="ExternalInput")
    cT1 = nc.dram_tensor("cT1", [5, VPAD], F32, kind="ExternalInput")
    W1a = nc.dram_tensor("W1a", [HID, MID], F32, kind="ExternalInput")
    W1b = nc.dram_tensor("W1b", [5, MID], F32, kind="ExternalInput")
    w1dt = nc.dram_tensor("w1dt", [128, MID], F32, kind="ExternalInput")
    W2t = nc.dram_tensor("W2t", [128, MID], F32, kind="ExternalInput")
    iotaF = nc.dram_tensor("iotaF", [128, 64], BF16, kind="ExternalInput")
    x4r = nc.dram_tensor("x4r", [VPAD, 4], F32, kind="ExternalInput")
    xpair = nc.dram_tensor("xpair", [RTP, 64], F32, kind="ExternalInput")
    rli = nc.dram_tensor("rli", [nwin, 128, BLKE // 16], I16,
                         kind="ExternalInput")
    cpi = nc.dram_tensor("cpi", [nwin, 128, BLKE // 16], I16,
                         kind="ExternalInput")
    par = nc.dram_tensor("par", [nwin, 128, G], F32, kind="ExternalInput")
    rrel = nc.dram_tensor("rrel", [nwin, 128, G], BF16, kind="ExternalInput")
    outT = nc.dram_tensor("out", [VPAD, 4], F32, kind="ExternalOutput")
    gx = nc.dram_tensor("gx", [VPAD, 128], F32)

    gx_pcf = gx.ap().rearrange("(c p) f -> p c f", p=128)
    x4r_pcf = x4r.ap().rearrange("(c p) f -> p c f", p=128)
    out_pcf = outT.ap().rearrange("(c p) f -> p c f", p=128)

    with tile.TileContext(nc) as tc, ExitStack() as ctx:
        const = ctx.enter_context(tc.tile_pool(name="const", bufs=1))
        p0 = ctx.enter_context(tc.tile_pool(name="p0", bufs=3))
        psum = ctx.enter_context(tc.tile_pool(name="psum", bufs=4,
                                              space="PSUM"))

        W1a_s = const.tile([HID, MID], F32)
        nc.sync.dma_start(W1a_s[:], W1a.ap())
        W1b_s = const.tile([5, MID], F32)
        nc.sync.dma_start(W1b_s[:], W1b.ap())
        w1dt_s = const.tile([128, MID], F32)
        nc.sync.dma_start(w1dt_s[:], w1dt.ap())
        W2t_s = const.tile([128, MID], F32)
        nc.sync.dma_start(W2t_s[:], W2t.ap())
        iota_s = const.tile([128, 64], BF16)
        nc.sync.dma_start(iota_s[:], iotaF.ap())
        # SBUF-resident accumulator D: [128, nwin/2, 4] holds 64-node
        # windows interleaved: window w -> partitions (w%2)*64..+64,
        # free column w//2.
        Dacc = const.tile([128, nch, 4], F32)
        nc.vector.memset(Dacc[:], 0.0)

        # ---- phase 0: build gx table (f32: g | x | zeros)
        for c in range(nch):
            ht_s = p0.tile([HID, 128], F32, tag="ht")
            nc.sync.dma_start(ht_s[:], hT.ap()[:, c * 128:(c + 1) * 128])
            ct_s = p0.tile([5, 128], F32, tag="ct")
            nc.sync.dma_start(ct_s[:], cT1.ap()[:, c * 128:(c + 1) * 128])
            ps = psum.tile([128, MID], F32, tag="ps0")
            nc.tensor.matmul(out=ps[:], lhsT=ht_s[:], rhs=W1a_s[:],
                             start=True, stop=False)
            nc.tensor.matmul(out=ps[:], lhsT=ct_s[:], rhs=W1b_s[:],
                             start=False, stop=True)
            x4c = p0.tile([128, 4], F32, tag="x4c")
            nc.sync.dma_start(x4c[:], x4r_pcf[:, c, :])
            stg = p0.tile([128, 128], F32, tag="stg")
            nc.vector.tensor_copy(stg[:, 0:64], ps[:])
            nc.vector.tensor_copy(stg[:, 64:68], x4c[:])
            nc.vector.memset(stg[:, 68:128], 0.0)
            nc.sync.dma_start(gx_pcf[:, c, :], stg[:])

        tc.strict_bb_all_engine_barrier()

        # ---- phase 1: super-blocks of SBW 64-node windows
        pidx = ctx.enter_context(tc.tile_pool(name="pidx", bufs=2))
        pg = ctx.enter_context(tc.tile_pool(name="pg", bufs=2))
        pmm = ctx.enter_context(tc.tile_pool(name="pmm", bufs=2))
        pt = ctx.enter_context(tc.tile_pool(name="pt", bufs=2))
        pm = ctx.enter_context(tc.tile_pool(name="pm", bufs=3))
        pps = ctx.enter_context(tc.tile_pool(name="pps", bufs=4,
                                             space="PSUM"))

        for sb in range(NSB):
            w0 = sb * SBW
            rl_s = pidx.tile([128, SBE // 16], I16, tag="rl")
            nc.sync.dma_start(
                rl_s[:].rearrange("p (w s) -> p w s", w=SBW),
                rli.ap()[w0:w0 + SBW].rearrange("w p s -> p w s"))
            cp_s = pidx.tile([128, SBE // 16], I16, tag="cp")
            nc.sync.dma_start(
                cp_s[:].rearrange("p (w s) -> p w s", w=SBW),
                cpi.ap()[w0:w0 + SBW].rearrange("w p s -> p w s"))
            par_s = pidx.tile([128, SBW, G], F32, tag="par")
            nc.sync.dma_start(par_s[:],
                              par.ap()[w0:w0 + SBW].rearrange("w p g -> p w g"))
            rr_s = pidx.tile([128, SBW, G], BF16, tag="rr")
            nc.sync.dma_start(rr_s[:],
                              rrel.ap()[w0:w0 + SBW].rearrange("w p g -> p w g"))

            gxr = pg.tile([128, SBW * G, 68], F32, tag="gxr")
            dma_gather_raw(nc.gpsimd, gxr[:], gx.ap()[:, 0:68], rl_s[:],
                           SBE, SBE, elem_size=68, elem_step=128,
                           single_packet=False)
            xcp = pg.tile([128, SBW * G, 8], F32, tag="xcp")
            dma_gather_raw(nc.gpsimd, xcp[:], xpair.ap()[:, 0:8], cp_s[:],
                           SBE, SBE, elem_size=8, elem_step=64,
                           single_packet=False)

            # one-hot M for the whole super-block (bf16)
            M_sb = pmm.tile([128, SBW * G, 64], BF16, tag="msb")
            nc.vector.tensor_tensor(
                out=M_sb[:],
                in0=rr_s[:].rearrange("p w g -> p (w g)")[:, :, None]
                    .to_broadcast([128, SBW * G, 64]),
                in1=iota_s[:][:, None, :].to_broadcast([128, SBW * G, 64]),
                op=ALU.is_equal)

            # parity select: xc4 = lo + (hi - lo) * par
            tsel = pt.tile([128, SBW * G, 4], F32, tag="tsel")
            nc.vector.tensor_tensor(out=tsel[:], in0=xcp[:, :, 4:8],
                                    in1=xcp[:, :, 0:4], op=ALU.subtract)
            tsel2 = pt.tile([128, SBW * G, 4], F32, tag="tsel2")
            nc.vector.tensor_tensor(
                out=tsel2[:], in0=tsel[:],
                in1=par_s[:].rearrange("p w g -> p (w g)")[:, :, None]
                    .to_broadcast([128, SBW * G, 4]),
                op=ALU.mult)
            xc4 = pt.tile([128, SBW * G, 4], F32, tag="xc4")
            nc.vector.tensor_tensor(out=xc4[:], in0=tsel2[:],
                                    in1=xcp[:, :, 0:4], op=ALU.add)

            # dist pipeline (f32)
            diff = pt.tile([128, SBW * G, 4], F32, tag="diff")
            nc.vector.tensor_tensor(out=diff[:], in0=xc4[:],
                                    in1=gxr[:, :, 64:68], op=ALU.subtract)
            sq = pt.tile([128, SBW * G, 4], F32, tag="sq")
            nc.vector.tensor_tensor(out=sq[:], in0=diff[:], in1=diff[:],
                                    op=ALU.mult)
            d2 = pt.tile([128, SBW * G], F32, tag="d2")
            nc.vector.tensor_reduce(d2[:], sq[:], mybir.AxisListType.X,
                                    ALU.add)
            d2c = pt.tile([128, SBW * G], F32, tag="d2c")
            nc.vector.tensor_scalar_max(d2c[:], d2[:], EPS2)
            dist = pt.tile([128, SBW * G], F32, tag="dist")
            nc.scalar.sqrt(dist[:], d2c[:])
            rinv = pt.tile([128, SBW * G], F32, tag="rinv")
            nc.vector.reciprocal(rinv[:], dist[:])

            # MLP (f32, edge-major); am-mult on GPSIMD
            uv = pt.tile([128, SBW * G, 4], BF16, tag="uv")
            for c0 in range(0, SBW * G, CS):
                c1 = min(c0 + CS, SBW * G)
                cw = c1 - c0
                cs = slice(c0, c1)
                pre = pm.tile([128, CS, MID], F32, tag="pre", name="pre")[:, :cw, :]
                nc.vector.tensor_tensor(
                    out=pre[:],
                    in0=w1dt_s[:][:, None, :].to_broadcast([128, cw, MID]),
                    in1=dist[:, cs][:, :, None].to_broadcast([128, cw, MID]),
                    op=ALU.mult)
                pre2 = pm.tile([128, CS, MID], F32, tag="pre2", name="pre2")[:, :cw, :]
                nc.vector.tensor_tensor(out=pre2[:], in0=pre[:],
                                        in1=gxr[:, cs, 0:64], op=ALU.add)
                sil = pm.tile([128, CS, MID], F32, tag="sil", name="sil")[:, :cw, :]
                nc.scalar.activation(sil[:], pre2[:], AF.Silu)
                am = pm.tile([128, CS, MID], F32, tag="am", name="am")[:, :cw, :]
                nc.gpsimd.tensor_tensor(
                    out=am[:], in0=sil[:],
                    in1=W2t_s[:][:, None, :].to_broadcast([128, cw, MID]),
                    op=ALU.mult)
                al = pm.tile([128, CS], F32, tag="al", name="al")[:, :cw]
                nc.vector.tensor_reduce(al[:], am[:], mybir.AxisListType.X,
                                        ALU.add)
                if b2val != 0.0:
                    al2 = pm.tile([128, CS], F32, tag="al2", name="al2")[:, :cw]
                    nc.vector.tensor_scalar_add(al2[:], al[:], b2val)
                    al = al2
                sr = pm.tile([128, CS], F32, tag="sr", name="sr")[:, :cw]
                nc.vector.tensor_tensor(out=sr[:], in0=al[:],
                                        in1=rinv[:, cs], op=ALU.mult)
                nc.vector.tensor_tensor(
                    out=uv[:, cs, :], in0=diff[:, cs, :],
                    in1=sr[:][:, :, None].to_broadcast([128, cw, 4]),
                    op=ALU.mult)

            # scatter: per window, accumulate into PSUM via one-hot matmuls
            for wi in range(SBW):
                w = w0 + wi
                dps = pps.tile([64, 4], F32, tag="dps")
                for g in range(G):
                    gc = wi * G + g
                    nc.tensor.matmul(out=dps[:], lhsT=M_sb[:, gc, :],
                                     rhs=uv[:, gc, :], start=(g == 0),
                                     stop=(g == G - 1))
                nc.vector.tensor_add(
                    out=Dacc[(w % 2) * 64:(w % 2) * 64 + 64, w // 2, :],
                    in0=Dacc[(w % 2) * 64:(w % 2) * 64 + 64, w // 2, :],
                    in1=dps[:])

        # ---- phase 2: normalize
        p2 = ctx.enter_context(tc.tile_pool(name="p2", bufs=1))
        sq2 = p2.tile([128, nch, 4], F32)
        nc.vector.tensor_tensor(out=sq2[:], in0=Dacc[:], in1=Dacc[:],
                                op=ALU.mult)
        n2 = p2.tile([128, nch], F32)
        nc.vector.tensor_reduce(n2[:], sq2[:], mybir.AxisListType.X, ALU.add)
        n2c = p2.tile([128, nch], F32)
        nc.vector.tensor_scalar_max(n2c[:], n2[:], EPS2)
        nrm = p2.tile([128, nch], F32)
        nc.scalar.sqrt(nrm[:], n2c[:])
        rn = p2.tile([128, nch], F32)
        nc.vector.reciprocal(rn[:], nrm[:])
        ot = p2.tile([128, nch, 4], F32)
        nc.vector.tensor_tensor(
            out=ot[:], in0=Dacc[:],
            in1=rn[:][:, :, None].to_broadcast([128, nch, 4]), op=ALU.mult)
        nc.sync.dma_start(out_pcf, ot[:])

    nc.compile()
    return nc


# ---------------------------------------------------------------- entry point

def _ensure_axon_ntff_hook():
    """The container ships libaxon_pjrt's NTFF-profile ABI but not the tiny
    antenv.axon_hooks registry bass_utils reads. Synthesize it."""
    try:
        import antenv
        try:
            from antenv import axon_hooks  # noqa: F401
            return True
        except ImportError:
            pass
        import types

        mod = types.ModuleType("antenv.axon_hooks")
        mod._hook = None
        mod.set_axon_ntff_profile_hook = lambda h: setattr(mod, "_hook", h)
        mod.get_axon_ntff_profile_hook = lambda: mod._hook
        sys.modules["antenv.axon_hooks"] = mod
        antenv.axon_hooks = mod
        from trn_agent_boot.trn_boot import _ntff_profile_via_ctypes
        hook = _ntff_profile_via_ctypes("/opt/axon/libaxon_pjrt.so")
        mod.set_axon_ntff_profile_hook(hook)
        from concourse import bass_utils as bu
        bu.upload_artifacts = lambda tmpdir: tmpdir  # no S3 in container
        return hook is not None
    except Exception as e:  # pragma: no cover
        print(f"ntff hook setup failed: {e}")
        return False


_CACHE = {}
LAST_RESULT = None  # BassKernelResults of the most recent run (for test.py)

REAL_CFG = Cfg(V=50000, VPC=6250, VPAD=6272, RTP=25088, G=0)


def run(inputs, cfg, trace=False):
    global LAST_RESULT
    in_maps, cfg2, b2val = prep_inputs(
        inputs["h"], inputs["x"], inputs["curvature"], inputs["edge_index"],
        inputs["W1"], inputs["b1"], inputs["W2"], inputs["b2"], cfg)
    key = (cfg2, b2val)
    if key not in _CACHE:
        _CACHE[key] = build_graph(cfg2, b2val)
    nc = _CACHE[key]
    if trace and not _ensure_axon_ntff_hook():
        trace = False
    res = run_bass_kernel_spmd(nc, in_maps, core_ids=list(range(NCORES)),
                               trace=trace)
    LAST_RESULT = res
    V, VPC = cfg.V, cfg.VPC
    out = np.empty((V, 3), np.float32)
    for k in range(NCORES):
        out[k * VPC:(k + 1) * VPC] = res.results[k]["out"][:VPC, 0:3]
    return out


def kernel(**inputs) -> np.ndarray:
    trace = bool(os.environ.get("KERNEL_TRACE"))
    return run(inputs, REAL_CFG, trace=trace)
